# revision 4
# baseline (speedup 1.0000x reference)
import sys
sys.path.insert(0, "/opt/trn_rl_repo")
import numpy as np
import concourse.bass as bass
import concourse.bacc as bacc
import concourse.mybir as mybir
import concourse.tile as tile
from concourse.bass_utils import run_bass_kernel_spmd

# Problem constants (hardcoded per contract)
N = 20000
T = 20
D = 64
H = 64
W = 3
NCORES = 8
NPAD = 24576            # 8 * 3072
PER_CORE = 3072         # padded per-core stocks
C = 512                 # chunk size
NPAIR = 3               # pairs of chunks per core (6 chunks)
dt = mybir.dt

_cache = {}


def _build_program():
    """Bass program: GRU over T steps for 3 weeks x 3 chunk-pairs of 1024 stocks.

    Layouts per (week, pair):
      xh_A/xh_B [128, 21*C]: rows 0:64 x features (slot t = x_t), rows 64:128
        h state (slot t = h_{t-1});  hs [128, 21*C]: packed h (A on 0:64,
        B on 64:128), slot t = h_{t-1}.
    Gate matmuls K=128 (stacked [x;h]) with M=64, col-tiled A->psum[0:64],
    B->psum[64:128] (tile_position=(0,64)).  fp32r for full-rate fp32.
    """
    nc = bacc.Bacc("TRN2", target_bir_lowering=False, debug=False,
                   num_devices=NCORES)
    SLOTS = 21 * C
    x_in = nc.declare_dram_parameter("x", [W, NPAIR, 2, 64, T * C], dt.float32,
                                     isOutput=False)
    wl_in = nc.declare_dram_parameter("wl", [128, W * 4 * 64], dt.float32,
                                      isOutput=False)
    bl_in = nc.declare_dram_parameter("bl", [128, W * 4], dt.float32,
                                      isOutput=False)
    hs_out = nc.declare_dram_parameter("hs", [W, NPAIR, 128, T * C], dt.float32,
                                       isOutput=True)
    f32r = dt.float32r
    AF = mybir.ActivationFunctionType
    OP = mybir.AluOpType

    with tile.TileContext(nc) as tc:
        with tc.tile_pool(name="wpool", bufs=1) as wpool, \
             tc.tile_pool(name="xh", bufs=1) as xhp, \
             tc.tile_pool(name="hsp", bufs=1) as hsp, \
             tc.tile_pool(name="gate", bufs=2) as gp, \
             tc.tile_pool(name="psum", bufs=2, space="PSUM") as pp:
            wl = wpool.tile([128, W * 4 * 64], dt.float32)
            bl = wpool.tile([128, W * 4], dt.float32)
            nc.sync.dma_start(out=wl[:], in_=wl_in[:, :])
            nc.sync.dma_start(out=bl[:], in_=bl_in[:, :])

            for w in range(W):
                for p in range(NPAIR):
                    xh_A = xhp.tile([128, SLOTS], dt.float32, tag="xha")
                    xh_B = xhp.tile([128, SLOTS], dt.float32, tag="xhb")
                    hs = hsp.tile([128, SLOTS], dt.float32, tag="hs")
                    nc.sync.dma_start(out=xh_A[0:64, 0:T * C], in_=x_in[w, p, 0])
                    nc.sync.dma_start(out=xh_B[0:64, 0:T * C], in_=x_in[w, p, 1])
                    nc.vector.memset(xh_A[64:128, 0:C], 0.0)
                    nc.vector.memset(xh_B[64:128, 0:C], 0.0)
                    nc.vector.memset(hs[:, 0:C], 0.0)
                    for t in range(T):
                        sl = slice(t * C, (t + 1) * C)
                        sl1 = slice((t + 1) * C, (t + 2) * C)
                        ps = []
                        for g in range(4):
                            pg = pp.tile([128, C], dt.float32, tag=f"g{g}")
                            lt = wl[:, (w * 4 + g) * 64:(w * 4 + g + 1) * 64]
                            nc.tensor.matmul(out=pg[0:64, :],
                                             lhsT=lt,
                                             rhs=xh_A[:, sl],
                                             start=True, stop=True)
                            nc.tensor.matmul(out=pg[64:128, :],
                                             lhsT=lt,
                                             rhs=xh_B[:, sl],
                                             start=True, stop=True,
                                             tile_position=(0, 64))
                            ps.append(pg)
                        r = gp.tile([128, C], dt.float32, tag="r")
                        z = gp.tile([128, C], dt.float32, tag="z")
                        v = gp.tile([128, C], dt.float32, tag="v")
                        wo = gp.tile([128, C], dt.float32, tag="wo")
                        c_ = gp.tile([128, C], dt.float32, tag="c")
                        s_ = gp.tile([128, C], dt.float32, tag="s")
                        t_ = gp.tile([128, C], dt.float32, tag="t")
                        nc.scalar.activation(out=r[:], in_=ps[0][:], func=AF.Sigmoid,
                                             bias=bl[:, (w * 4 + 0):(w * 4 + 1)])
                        nc.scalar.activation(out=z[:], in_=ps[1][:], func=AF.Sigmoid,
                                             bias=bl[:, (w * 4 + 1):(w * 4 + 2)])
                        # v = (hn + b_hn) * r ; wo = (xn + b_in) + v
                        nc.vector.scalar_tensor_tensor(
                            out=v[:], in0=ps[2][:], scalar=bl[:, (w * 4 + 2):(w * 4 + 3)],
                            in1=r[:], op0=OP.add, op1=OP.mult)
                        nc.vector.scalar_tensor_tensor(
                            out=wo[:], in0=ps[3][:], scalar=bl[:, (w * 4 + 3):(w * 4 + 4)],
                            in1=v[:], op0=OP.add, op1=OP.add)
                        nc.scalar.activation(out=c_[:], in_=wo[:], func=AF.Tanh)
                        nc.vector.tensor_sub(out=s_[:], in0=hs[:, sl], in1=c_[:])
                        nc.vector.tensor_mul(out=t_[:], in0=z[:], in1=s_[:])
                        nc.vector.tensor_add(out=hs[:, sl1], in0=c_[:], in1=t_[:])
                        if t < T - 1:
                            nc.sync.dma_start(out=xh_A[64:128, sl1], in_=hs[0:64, sl1])
                            nc.sync.dma_start(out=xh_B[64:128, sl1], in_=hs[64:128, sl1])
                    nc.sync.dma_start(out=hs_out[w, p], in_=hs[:, C:SLOTS])
    nc.compile()
    return nc


def _prep_inputs(x0, x1, x2, gru_wih, gru_whh, gru_bih, gru_bhh):
    xs = np.stack([x0, x1, x2])  # [W, N, T, D]
    xpad = np.zeros((W, NPAD, T, D), np.float32)
    xpad[:, :N] = xs
    # per-core x: [W, NPAIR, 2, 64, T*C]
    in_maps = []
    # weights: lhsT per gate: [K=128, M=64]
    wl = np.zeros((128, W * 4 * 64), np.float32)
    bl = np.zeros((128, W * 4), np.float32)
    for w in range(W):
        wih, whh = gru_wih[w], gru_whh[w]        # [3H, D], [3H, H]
        bih, bhh = gru_bih[w], gru_bhh[w]
        for g, (top, bot, bias) in enumerate([
                (wih[0:64], whh[0:64], bih[0:64] + bhh[0:64]),          # r
                (wih[64:128], whh[64:128], bih[64:128] + bhh[64:128]),  # z
                (np.zeros((64, 64), np.float32), whh[128:192], bhh[128:192]),  # hn
                (wih[128:192], np.zeros((64, 64), np.float32), bih[128:192]),  # xn
        ]):
            col = (w * 4 + g) * 64
            wl[0:64, col:col + 64] = top.T
            wl[64:128, col:col + 64] = bot.T
            bl[0:64, w * 4 + g] = bias
            bl[64:128, w * 4 + g] = bias
    for cid in range(NCORES):
        sl = xpad[:, cid * PER_CORE:(cid + 1) * PER_CORE]  # [W, 3072, T, D]
        xc = np.zeros((W, NPAIR, 2, 64, T * C), np.float32)
        for p in range(NPAIR):
            for hfl in range(2):
                blk = sl[:, (p * 2 + hfl) * C:(p * 2 + hfl + 1) * C]  # [W,C,T,D]
                xc[:, p, hfl] = blk.transpose(0, 3, 2, 1).reshape(W, 64, T * C)
        in_maps.append({"x": xc, "wl": wl, "bl": bl})
    return in_maps


def _np_attn(seq, w, b):
    st = np.swapaxes(seq, 1, 2)
    e = st @ w.T + b
    e = e - e.max(-1, keepdims=True)
    p = np.exp(e)
    p = p / p.sum(-1, keepdims=True)
    return np.sum(np.swapaxes(p, 1, 2) * seq, axis=1)


def kernel(x0, x1, x2, gru_wih, gru_whh, gru_bih, gru_bhh, att_w, att_b,
           ww_w, ww_b, gat_w, gat_att_src, gat_att_dst, gat_b,
           fus_w, fus_b, reg_w, reg_b, cls_w, cls_b, edge_index):
    if "nc" not in _cache:
        _cache["nc"] = _build_program()
    nc = _cache["nc"]
    in_maps = _prep_inputs(x0, x1, x2, gru_wih, gru_whh, gru_bih, gru_bhh)
    res = run_bass_kernel_spmd(nc, in_maps, list(range(NCORES)))
    _cache["exec_ns"] = res.exec_time_ns
    # reassemble hs: [W, N, T, H]
    hs = np.zeros((W, NPAD, T, H), np.float32)
    for cid in range(NCORES):
        h = res.results[cid]["hs"]  # [W, NPAIR, 128, T*C]
        h = h.reshape(W, NPAIR, 128, T, C)
        for p in range(NPAIR):
            base = cid * PER_CORE + p * 2 * C
            hs[:, base:base + C] = h[:, p, 0:64].transpose(0, 3, 2, 1)
            hs[:, base + C:base + 2 * C] = h[:, p, 64:128].transpose(0, 3, 2, 1)
    hs = hs[:, :N]  # [W, N, T, H]

    # host: attention blocks + GAT + fusion (numpy)
    emb = np.stack([_np_attn(hs[w], att_w[w], att_b[w]) for w in range(W)])
    emb = np.swapaxes(emb, 0, 1)                  # (N, W, H)
    weekly = _np_attn(emb, ww_w, ww_b)            # (N, H)

    xg = weekly @ gat_w.T
    loops = np.arange(N, dtype=edge_index.dtype)
    src = np.concatenate([edge_index[0], loops])
    dst = np.concatenate([edge_index[1], loops])
    a = xg @ gat_att_src + 0.0
    ad = xg @ gat_att_dst
    alpha = a[src] + ad[dst]
    alpha = np.where(alpha > 0, alpha, 0.2 * alpha)
    amax = np.full(N, -np.inf, np.float32)
    np.maximum.at(amax, dst, alpha)
    ex = np.exp(alpha - amax[dst])
    den = np.bincount(dst, weights=ex, minlength=N)
    coef = (ex / den[dst]).astype(np.float32)
    cat = np.zeros((N, H), np.float32)
    wsrc = coef[:, None] * xg[src]
    for f in range(H):
        cat[:, f] = np.bincount(dst, weights=wsrc[:, f], minlength=N)
    cat = cat + gat_b

    fus = np.concatenate([weekly, cat], axis=-1) @ fus_w.T + fus_b
    fus = np.maximum(fus, 0.0)
    reg = np.ravel(fus @ reg_w.T + reg_b)
    cls = np.ravel(1.0 / (1.0 + np.exp(-(fus @ cls_w.T + cls_b))))
    return (reg.astype(np.float32), cls.astype(np.float32))
